# revision 47
# baseline (speedup 1.0000x reference)
"""DimensionWiseMINE on 8 Trainium2 NeuronCores.

Sharding: stage-1 gene projector x@Wg1 is contraction-sharded (XD/8 per core)
with a chunked bf16 AllReduce of the pre-activation overlapping the matmul;
everything downstream of h is expert-parallel over the D=64 per-dim nets
(8 dims per core). The batch permutation is applied on the host to z (the
batch sum is permutation invariant), so no on-device gather is needed.

Layouts are feature-major ("transposed": [feature_partition, batch_free])
throughout, so no on-device transposes are needed.

ELU is computed as  elu(y)+1 = max(y+1, min(exp(y), 1))  -- one ACT op
(exp with bias -1 reading y+1) plus one fused DVE scalar_tensor_tensor op.
The +1 shifts are absorbed into the next layer's bias via weight row/col
sums, and all biases ride the matmuls as extra contraction rows (hi/lo bf16
split for accuracy) or ACT bias slots.
"""

import numpy as np
import ml_dtypes

import concourse.bass as bass
import concourse.bacc as bacc
import concourse.tile as tile
from concourse import mybir
from concourse.bass_utils import run_bass_kernel_spmd

NCORES = 8
B, XD, PD, D, H, G1 = 512, 32768, 512, 64, 128, 1024
KSH = XD // NCORES   # 4096 contraction slice per core
DL = D // NCORES     # 8 local dims per core
NKDR = KSH // 256    # 16 DoubleRow k-steps for MM1
NG = 4               # MM1 m-groups (each 2 m-tiles of g1) / AllReduce chunks
XSC = 8.0            # fp8 scale on x
WSC = 64.0           # fp8 scale on Wg1
F32 = mybir.dt.float32
BF16 = mybir.dt.bfloat16
F8 = mybir.dt.float8e4
BF = ml_dtypes.bfloat16
F8NP = ml_dtypes.float8_e4m3
AF = mybir.ActivationFunctionType
OP = mybir.AluOpType
DRM = mybir.MatmulPerfMode.DoubleRow

_cache = {}


def _build(sim=False, stage=3):
    nc = bacc.Bacc("TRN2", target_bir_lowering=False, debug=False,
                   num_devices=1 if sim else NCORES)

    # x: chunk q holds DR k-steps 4q..4q+3: [128, kc, i, b] (fp8, scaled x8)
    xt_d = nc.declare_dram_parameter("xt", [4, 128, 4 * 2 * B], F8, isOutput=False)
    # wg1: per (m-pair g, k-octet h): [128, k8, i, m2, 128] (fp8, scaled x64)
    wg1_d = nc.declare_dram_parameter("wg1", [NG, 2, 128, 8 * 2 * 256], F8, isOutput=False)
    f32p_d = nc.declare_dram_parameter("f32p", [128, 12], F32, isOutput=False)
    w2all_d = nc.declare_dram_parameter("w2all", [128, 9216], F8, isOutput=False)

    bfp_d = nc.declare_dram_parameter("bfp", [3, 12288], BF16, isOutput=False)
    w3_d = nc.declare_dram_parameter("w3", [128, DL], BF16, isOutput=False)
    out_d = nc.declare_dram_parameter("out", [384], F32, isOutput=True)

    GROUPS = [[0, 1], [2, 3], [4, 5], [6, 7]]
    aring = [nc.dram_tensor(f"arin{g}", [128, len(grp) * B], F8)
             for g, grp in enumerate(GROUPS)]
    aroutg = [nc.dram_tensor(f"arout{g}", [128, len(grp) * B], F8,
                             addr_space="Shared")
              for g, grp in enumerate(GROUPS)]

    with tile.TileContext(nc) as tc:
        with (
            tc.tile_pool(name="wg1p", bufs=2) as wg1p,
            tc.tile_pool(name="castp", bufs=4) as castp,
            tc.tile_pool(name="consts", bufs=1) as cst,
            tc.tile_pool(name="work", bufs=1) as wk,
            tc.tile_pool(name="ep", bufs=6) as ep,
            tc.tile_pool(name="ap_", bufs=10) as app,
            tc.tile_pool(name="mep", bufs=4) as mep,
        ):
            # ---- startup DMAs in priority order: first MM1 operands,
            # then the rest of x / Wg1, then constants ----
            xtt = [cst.tile([128, 4, 2, B], F8, name=f"xts_{i}") for i in range(4)]
            wg1h = [[wg1p.tile([128, 8, 2, 2, 128], F8,
                               name=f"wg_{g}_{h}") for h in range(2)]
                    for g in range(NG)]
            # interleave so the first matmuls' operands land first
            nc.sync.dma_start(wg1h[0][0][:, 0:2], wg1_d[0, 0][:, 0:1024])
            nc.sync.dma_start(xtt[0][:, 0:2], xt_d[0][:, 0:2048])
            nc.sync.dma_start(wg1h[0][0][:, 2:8], wg1_d[0, 0][:, 1024:4096])
            nc.sync.dma_start(xtt[0][:, 2:4], xt_d[0][:, 2048:4096])
            nc.sync.dma_start(xtt[1][:], xt_d[1])
            nc.sync.dma_start(wg1h[0][1][:], wg1_d[0, 1])
            nc.sync.dma_start(xtt[2][:], xt_d[2])
            nc.sync.dma_start(xtt[3][:], xt_d[3])
            for g in range(1, NG):
                for h in range(2):
                    nc.sync.dma_start(wg1h[g][h][:], wg1_d[g, h])
            w2all = cst.tile([128, 9216], F8)
            nc.sync.dma_start(w2all[:], w2all_d[:])
            wg2sb = w2all[:, 0:4096]
            wht = w2all[:, 4096:8192]
            w2t = w2all[:, 8192:9216]
            # ---- constants: packed into 3 DMAs (f32 pack, bf16 pack, w3)
            # plus the big fp8 stage-2 weights late ----
            f32p = cst.tile([128, 12], F32)
            nc.scalar.dma_start(f32p[:], f32p_d[:])
            bg1c = f32p[:, 0:8]
            cvec4 = f32p[:, 8:12]
            bfp = cst.tile([3, 12288], BF16)
            nc.scalar.dma_start(bfp[:], bfp_d[:])
            b2r = bfp[0:2, 0:512]
            l1b = bfp[:, 512:1536]
            zj = bfp[:, 1536:5632]
            zd = bfp[0:1, 5632:9728]
            l2b = bfp[0:2, 9728:10752]
            ones2 = bfp[0:2, 10752:11264]
            bg1r = bfp[0:1, 11264:12288]
            w3t = cst.tile([128, DL], BF16)

            # warm the exp table set early so the ~2.7us load hides under MM1
            wz0 = cst.tile([128, 1], F32)
            nc.vector.memset(wz0[:], 0.0)
            wz1 = cst.tile([128, 1], F32)
            nc.scalar.activation(wz1[:], wz0[:], AF.Exp)
            neg1 = cst.tile([128, 1], F32)
            nc.vector.memset(neg1[:], -1.0)


            h1g = [wk.tile([128, len(grp) * B], F8, name=f"h1g_{g}")
                   for g, grp in enumerate(GROUPS)]
            e1m = [wk.tile([128, B], BF16, name=f"e1m_{m}") for m in range(8)]
            a1m = [wk.tile([128, B], BF16, name=f"a1m_{m}") for m in range(8)]
            htm = [wk.tile([128, B], BF16, name=f"htm_{mt}") for mt in range(4)]

            with (
                tc.tile_pool(name="ps1", bufs=4, space="PSUM") as ps1,
                tc.tile_pool(name="ps2p", bufs=4, space="PSUM") as ps2p,
            ):
                ps2 = [ps2p.tile([128, B], F32, tag="psmm2", name=f"psmm2_{i}")
                       for i in range(4)]
                # ---- MM1 k-contiguous per m-group + chunked AllReduce.
                # Last pair split into single-m-tile groups so the final
                # (exposed) AllReduce is half the size with half the tail.
                pend = []
                for gi, grp in enumerate(GROUPS):
                    pair = grp[0] // 2
                    gw = len(grp)
                    pg = {m: ps1.tile([128, B], F32, tag="psmm1",
                                      name=f"ps1_{m}") for m in grp}
                    for kt in range(NKDR):
                        wgt = wg1h[pair][kt // 8]
                        ko = kt % 8
                        xsrc = xtt[kt // 4]
                        kc = kt % 4
                        for m in grp:
                            jc = m % 2
                            nc.tensor.matmul(
                                pg[m][:],
                                wgt[:, ko, :, jc],
                                xsrc[:, kc],
                                start=(kt == 0), stop=(kt == NKDR - 1),
                                perf_mode=DRM)
                    for mi, m in enumerate(grp):
                        cp = castp.tile([128, B], F8, tag="cast", name=f"cp_{m}")
                        nc.vector.tensor_scalar(cp[:], pg[m][:],
                                                1.0 / (XSC * WSC),
                                                bg1c[:, m:m + 1],
                                                OP.mult, OP.add)
                        nc.scalar.dma_start(
                            aring[gi][:, mi * B:(mi + 1) * B], cp[:])
                    if stage < 2:
                        continue
                    if sim:
                        nc.gpsimd.dma_start(aroutg[gi][:], aring[gi][:])
                    else:
                        nc.gpsimd.collective_compute(
                            "AllReduce", OP.add,
                            replica_groups=[list(range(NCORES))],
                            ins=[aring[gi][:]], outs=[aroutg[gi][:]],
                        )
                    nc.sync.dma_start(h1g[gi][:], aroutg[gi][:])
                    pend.append((gi, grp))
                    if gi >= 1:
                        pgi, pgrp = pend.pop(0)
                        for mi, m in enumerate(pgrp):
                            h1s = h1g[pgi][:, mi * B:(mi + 1) * B]
                            nc.scalar.activation(e1m[m][:], h1s,
                                                 AF.Exp, bias=neg1[:])
                            nc.vector.scalar_tensor_tensor(a1m[m][:],
                                                           e1m[m][:],
                                                           1.0, h1s,
                                                           OP.min, OP.max)
                    if gi == len(GROUPS) - 1:
                        for pgi, pgrp in pend:
                            for mi, m in enumerate(pgrp):
                                h1s = h1g[pgi][:, mi * B:(mi + 1) * B]
                                nc.scalar.activation(e1m[m][:], h1s,
                                                     AF.Exp, bias=neg1[:])
                                nc.vector.scalar_tensor_tensor(a1m[m][:],
                                                               e1m[m][:],
                                                               1.0, h1s,
                                                               OP.min, OP.max)
                        pend.clear()

                if stage >= 2:
                    nc.sync.dma_start(w3t[:], w3_d[:])
                    # MM2 partials kt 0..6 fill the PE gap while the last AR
                    # chunks land; kt=7 + elu emitted per-mt so htm[0]'s elu
                    # overlaps mt 1-3's matmuls.
                    for kt in range(7):
                        for mt in range(4):
                            nc.tensor.matmul(
                                ps2[mt][:],
                                wg2sb[:, kt * PD + mt * 128:kt * PD + (mt + 1) * 128],
                                a1m[kt][:],
                                start=(kt == 0), stop=False)
                        if kt == 0:
                            for mt in range(4):
                                nc.tensor.matmul(ps2[mt][:],
                                                 b2r[:, mt * 128:(mt + 1) * 128],
                                                 ones2[:], start=False,
                                                 stop=False)
                    for mt in range(4):
                        nc.tensor.matmul(
                            ps2[mt][:],
                            wg2sb[:, 7 * PD + mt * 128:7 * PD + (mt + 1) * 128],
                            a1m[7][:], start=False, stop=True)
                        eh = ep.tile([128, B], BF16, tag="escr", name=f"eh_{mt}")
                        nc.scalar.activation(eh[:], ps2[mt][:], AF.Exp, bias=neg1[:])
                        nc.vector.scalar_tensor_tensor(htm[mt][:], eh[:], 1.0,
                                                       ps2[mt][:], OP.min, OP.max)

            if stage >= 3:
                # ---- stage 2: per-dim nets, joint + marg ----
                rsumJ = wk.tile([128, DL], F32)
                ets4 = wk.tile([128, 4], F32)

                with (
                    tc.tile_pool(name="psL1", bufs=3, space="PSUM") as psL1,
                    tc.tile_pool(name="psL2", bufs=2, space="PSUM") as psL2,
                    tc.tile_pool(name="psm5", bufs=1, space="PSUM") as psm5,
                ):
                    def layer1J(d, nm):
                        """J pre: zj rows first (AR-wait fill), then hWh."""
                        pre = psL1.tile([128, B], F32, tag="psL1", name=f"pL1_{nm}")
                        nc.tensor.matmul(pre[:], l1b[:, d * H:(d + 1) * H],
                                         zj[:, d * B:(d + 1) * B],
                                         start=True, stop=False)
                        for kt in range(4):
                            nc.tensor.matmul(
                                pre[:],
                                wht[:, kt * DL * H + d * H:kt * DL * H + (d + 1) * H],
                                htm[kt][:],
                                start=False, stop=(kt == 3))
                        e = ep.tile([128, B], BF16, tag="escr", name=f"e_{nm}")
                        nc.scalar.activation(e[:], pre[:], AF.Exp, bias=neg1[:])
                        a = app.tile([128, B], BF16, tag="act", name=f"a_{nm}")
                        nc.vector.scalar_tensor_tensor(a[:], e[:], 1.0, pre[:],
                                                       OP.min, OP.max)
                        return pre, a

                    def layer1M(pre, d, nm):
                        """marg pre = joint pre + Wz*(zinv-z), in place."""
                        nc.tensor.matmul(pre[:], l1b[0:1, d * H:(d + 1) * H],
                                         zd[:, d * B:(d + 1) * B],
                                         start=False, stop=True,
                                         skip_group_check=True)
                        e = ep.tile([128, B], BF16, tag="escr", name=f"eM_{nm}")
                        nc.scalar.activation(e[:], pre[:], AF.Exp, bias=neg1[:])
                        a = app.tile([128, B], BF16, tag="act", name=f"aM_{nm}")
                        nc.vector.scalar_tensor_tensor(a[:], e[:], 1.0, pre[:],
                                                       OP.min, OP.max)
                        return a

                    def layer2pair(aJ, aM, d, nm):
                        pre = psL2.tile([128, 2 * B], F32, tag="psL2",
                                        name=f"pL2_{nm}")
                        for half, a in ((0, aJ), (1, aM)):
                            sl = slice(half * B, (half + 1) * B)
                            nc.tensor.matmul(pre[:, sl],
                                             w2t[:, d * H:(d + 1) * H], a[:],
                                             start=True, stop=False)
                            nc.tensor.matmul(pre[:, sl],
                                             l2b[:, d * H:(d + 1) * H],
                                             ones2[:], start=False, stop=True)
                        e = ep.tile([128, 2 * B], BF16, tag="escr2",
                                    name=f"e2_{nm}")
                        nc.scalar.activation(e[:], pre[:], AF.Exp, bias=neg1[:])
                        a2J = app.tile([128, B], BF16, tag="act", name=f"a2J_{nm}")
                        nc.vector.scalar_tensor_tensor(
                            a2J[:], e[:, 0:B], 1.0, pre[:, 0:B], OP.min, OP.max,
                            accum_out=rsumJ[:, d:d + 1])
                        a2M = app.tile([128, B], BF16, tag="act", name=f"a2M_{nm}")
                        nc.vector.scalar_tensor_tensor(
                            a2M[:], e[:, B:], 1.0, pre[:, B:], OP.min, OP.max)
                        return a2M

                    # Software pipeline, all cross-stage deps >= 1 iter old:
                    #   iter: L1J(d) | delta/eM/aM(d-1) | L2J(d-1)+L2M(d-2)
                    #         paired exp | L3(d-3) | mear per 2 dims.
                    preJ, aJs, aMs, aM2s = {}, {}, {}, {}
                    mrow = None
                    for it in range(DL + 3):
                        d = it
                        if d < DL:
                            # L1 joint: zj rows first, then hWh over htm
                            pre = psL1.tile([128, B], F32, tag="psL1",
                                            name=f"pL1_{d}")
                            nc.tensor.matmul(pre[:], l1b[:, d * H:(d + 1) * H],
                                             zj[:, d * B:(d + 1) * B],
                                             start=True, stop=False)
                            for kt in range(4):
                                nc.tensor.matmul(
                                    pre[:],
                                    wht[:, kt * DL * H + d * H:
                                        kt * DL * H + (d + 1) * H],
                                    htm[kt][:], start=False, stop=(kt == 3))
                            preJ[d] = pre
                            eJ = ep.tile([128, B], BF16, tag="escr",
                                         name=f"eJ_{d}")
                            nc.scalar.activation(eJ[:], pre[:], AF.Exp,
                                                 bias=neg1[:])
                            aJ = app.tile([128, B], BF16, tag="act",
                                          name=f"aJ_{d}")
                            nc.vector.scalar_tensor_tensor(aJ[:], eJ[:], 1.0,
                                                           pre[:], OP.min, OP.max)
                            aJs[d] = aJ
                        dm = it - 1
                        if 0 <= dm < DL:
                            # marg delta on J's psum (J reads finished last iter)
                            pre = preJ.pop(dm)
                            nc.tensor.matmul(pre[:], l1b[0:1, dm * H:(dm + 1) * H],
                                             zd[:, dm * B:(dm + 1) * B],
                                             start=False, stop=True,
                                             skip_group_check=True)
                            eM = ep.tile([128, B], BF16, tag="escr",
                                         name=f"eM_{dm}")
                            nc.scalar.activation(eM[:], pre[:], AF.Exp,
                                                 bias=neg1[:])
                            aM = app.tile([128, B], BF16, tag="act",
                                          name=f"aM_{dm}")
                            nc.vector.scalar_tensor_tensor(aM[:], eM[:], 1.0,
                                                           pre[:], OP.min, OP.max)
                            aMs[dm] = aM
                        # L2: J half for dim dm, M half for dim dm-1, one exp
                        dj, dmm = dm, dm - 1
                        havej, havem = 0 <= dj < DL, 0 <= dmm < DL
                        if havej or havem:
                            pre2 = psL2.tile([128, 2 * B], F32, tag="psL2",
                                             name=f"pL2_{it}")
                            if havej:
                                nc.tensor.matmul(pre2[:, 0:B],
                                                 w2t[:, dj * H:(dj + 1) * H],
                                                 aJs.pop(dj)[:],
                                                 start=True, stop=False)
                                nc.tensor.matmul(pre2[:, 0:B],
                                                 l2b[:, dj * H:(dj + 1) * H],
                                                 ones2[:], start=False, stop=True)
                            if havem:
                                nc.tensor.matmul(pre2[:, B:],
                                                 w2t[:, dmm * H:(dmm + 1) * H],
                                                 aMs.pop(dmm)[:],
                                                 start=True, stop=False)
                                nc.tensor.matmul(pre2[:, B:],
                                                 l2b[:, dmm * H:(dmm + 1) * H],
                                                 ones2[:], start=False, stop=True)
                            esl = (slice(0, 2 * B) if (havej and havem)
                                   else slice(0, B) if havej else slice(B, 2 * B))
                            e2 = ep.tile([128, 2 * B], BF16, tag="escr2",
                                         name=f"e2_{it}")
                            nc.scalar.activation(e2[:, esl], pre2[:, esl],
                                                 AF.Exp, bias=neg1[:])
                            if havej:
                                a2J = app.tile([128, B], BF16, tag="act",
                                               name=f"a2J_{dj}")
                                nc.vector.scalar_tensor_tensor(
                                    a2J[:], e2[:, 0:B], 1.0, pre2[:, 0:B],
                                    OP.min, OP.max,
                                    accum_out=rsumJ[:, dj:dj + 1])
                            if havem:
                                a2M = app.tile([128, B], BF16, tag="act",
                                               name=f"a2M_{dmm}")
                                nc.vector.scalar_tensor_tensor(
                                    a2M[:], e2[:, B:], 1.0, pre2[:, B:],
                                    OP.min, OP.max)
                                aM2s[dmm] = a2M
                        de = it - 3
                        if 0 <= de < DL:
                            aM2 = aM2s.pop(de)
                            if de % 2 == 0:
                                mrow = psm5.tile([128, B], F32, tag="psm5",
                                                 name=f"m5_{de}")
                            q = de % 2
                            nc.tensor.matmul(mrow[32 * q:32 * q + 1, :],
                                             w3t[:, de:de + 1],
                                             aM2[:], start=True, stop=True,
                                             skip_group_check=True)
                            if q == 1:
                                j = de // 2
                                mear = mep.tile([128, B], BF16, tag="mescr",
                                                name=f"me_{j}")
                                nc.scalar.activation(
                                    mear[:], mrow[:], AF.Exp,
                                    bias=cvec4[:, j:j + 1],
                                    accum_out=ets4[:, j:j + 1])
                                if j == 1:
                                    nc.sync.dma_start(out_d[0:128],
                                                      ets4[0:64, 0:2])
                                if j == 3:
                                    nc.sync.dma_start(out_d[128:256],
                                                      ets4[0:64, 2:4])

                # joint per-partition dot: jpp[k] = sum_d rsumJ[k,d]*W3[k,d]
                jsc = wk.tile([128, DL], F32)
                jpp = wk.tile([128, 1], F32)
                nc.vector.scalar_tensor_tensor(jsc[:], rsumJ[:], 1.0, w3t[:],
                                               OP.mult, OP.mult, accum_out=jpp[:])
                nc.sync.dma_start(out_d[256:384], jpp[:, 0:1])
    nc.compile()
    return nc


def _hilo(v):
    hi = v.astype(BF)
    lo = (v - hi.astype(np.float32)).astype(BF)
    return hi, lo


def _prep(x, z, perm, Wg1, bg1, Wg2, bg2, Wh, Wz, b1, W2, b2, W3, b3):
    """Build per-core input maps + host-side constants."""
    invperm = np.argsort(perm)
    zinv = z[invperm]                       # [B, D]
    bg2a = bg2 - Wg2.sum(axis=0)            # a1 shift correction
    b1a = b1 - Wh.sum(axis=1)               # [D, H] h~ shift correction
    b2a = b2 - W2.sum(axis=1)               # [D, H] a1 shift correction
    cj = b3 - W3.sum(axis=1)                # [D] a2 shift correction

    b2r_hi, b2r_lo = _hilo(1.0 + bg2a)
    ones2 = np.ones((2, B), BF)

    in_maps = []
    for c in range(NCORES):
        ksl = slice(c * KSH, (c + 1) * KSH)
        dsl = slice(c * DL, (c + 1) * DL)
        # xt[q][p, kc, i, b] = 8*x[b, c*KSH + (4q+kc)*256 + i*128 + p]
        xs = (x[:, ksl] * XSC).astype(F8NP)
        xt = np.ascontiguousarray(
            xs.T.reshape(4, 4, 2, 128, B).transpose(0, 3, 1, 2, 4)
            .reshape(4, 128, 4 * 2 * B))
        # wg1[g, h][p, k8, i, m2, mi] = 64*Wg1_c[(8h+k8)*256 + i*128 + p,
        #                                        (2g+m2)*128 + mi]
        ws = (Wg1[ksl] * WSC).astype(F8NP)
        wq = ws.reshape(NKDR, 2, 128, 8, 128).transpose(3, 0, 1, 2, 4)
        wg1 = np.ascontiguousarray(np.stack([
            np.stack([
                wq[2 * g:2 * g + 2, 8 * h:8 * h + 8]
                .transpose(3, 1, 2, 0, 4).reshape(128, 8 * 2 * 256)
                for h in range(2)])
            for g in range(NG)]))
        bg1c = ((bg1 + 1.0) / 8.0).astype(np.float32).reshape(8, 128).T
        cvec = np.zeros((128, 4), np.float32)
        for j in range(4):
            cvec[0, j] = cj[dsl][2 * j]
            cvec[32, j] = cj[dsl][2 * j + 1]
        f32p = np.concatenate([bg1c, cvec], axis=1).astype(np.float32)
        # wg2[p, kt*PD+m] = Wg2[kt*128+p, m]
        wg2 = np.ascontiguousarray(
            Wg2.reshape(8, 128, PD).transpose(1, 0, 2).reshape(128, 8 * PD)
        ).astype(F8NP)
        bg1row = (64.0 * (bg1 + 1.0)).astype(BF).reshape(1, G1)
        # wh[p, kt*DL*H + d*H+h] = Wh[dg, kt*128+p, h]
        wh = np.ascontiguousarray(
            Wh[dsl].transpose(1, 0, 2).reshape(4, 128, DL * H)
            .transpose(1, 0, 2).reshape(128, 4 * DL * H)).astype(F8NP)
        l1 = np.zeros((3, DL * H), np.float32)
        l1[0] = Wz[dsl].reshape(-1)
        v1hi, v1lo = _hilo((1.0 + b1a[dsl]).reshape(-1))
        l1b = np.stack([l1[0].astype(BF), v1hi, v1lo]).astype(BF)
        zjr = np.zeros((3, DL * B), np.float32)
        zjr[0] = z[:, dsl].T.reshape(-1)
        zjr[1:] = 1.0
        zdr = (zinv[:, dsl] - z[:, dsl]).T.reshape(1, DL * B)
        w2 = np.ascontiguousarray(
            W2[dsl].transpose(1, 0, 2).reshape(H, DL * H)
        ).astype(F8NP)
        v2hi, v2lo = _hilo((1.0 + b2a[dsl]).reshape(-1))
        l2b = np.stack([v2hi, v2lo])
        w3 = np.ascontiguousarray(W3[dsl].T).astype(BF)   # [H, DL]
        bfp = np.zeros((3, 12288), BF)
        bfp[0:2, 0:512] = np.stack([b2r_hi, b2r_lo])
        bfp[:, 512:1536] = l1b
        bfp[:, 1536:5632] = zjr.astype(BF)
        bfp[0:1, 5632:9728] = zdr.astype(BF)
        bfp[0:2, 9728:10752] = l2b
        bfp[0:2, 10752:11264] = ones2
        bfp[0:1, 11264:12288] = bg1row
        w2all = np.concatenate([wg2, wh, w2], axis=1)
        in_maps.append({
            "xt": xt, "wg1": wg1, "f32p": f32p, "w2all": w2all,
            "bfp": bfp, "w3": w3,
        })
    return in_maps, cj


def _combine(results, cj):
    """Host-side final reduction: 64 logs + means."""
    joint_sum = 0.0
    log_sum = 0.0
    for c in range(NCORES):
        o = results[c]["out"].astype(np.float64)
        joint_sum += o[256:384].sum() / B
        em = np.concatenate([o[0:128].reshape(64, 2),
                             o[128:256].reshape(64, 2)], axis=1)
        ets = em[0:64:32, :].T.reshape(-1)    # dims 0..7 local order
        log_sum += np.log(ets / B).sum()
    joint_sum += cj.astype(np.float64).sum()
    mi_sum = joint_sum - log_sum
    return np.float32(-mi_sum / D)


def kernel(x, z, perm, Wg1, bg1, Wg2, bg2, Wh, Wz, b1, W2, b2, W3, b3):
    args = (x, z, perm, Wg1, bg1, Wg2, bg2, Wh, Wz, b1, W2, b2, W3, b3)
    args = tuple(np.asarray(a) for a in args)
    in_maps, cj = _prep(*args)
    if "nc" not in _cache:
        _cache["nc"] = _build()
    r = run_bass_kernel_spmd(_cache["nc"], in_maps, list(range(NCORES)))
    return _combine(r.results, cj)



# revision 48
# speedup vs baseline: 1.0141x; 1.0141x over previous
"""DimensionWiseMINE on 8 Trainium2 NeuronCores.

Sharding: stage-1 gene projector x@Wg1 is contraction-sharded (XD/8 per core)
with a chunked bf16 AllReduce of the pre-activation overlapping the matmul;
everything downstream of h is expert-parallel over the D=64 per-dim nets
(8 dims per core). The batch permutation is applied on the host to z (the
batch sum is permutation invariant), so no on-device gather is needed.

Layouts are feature-major ("transposed": [feature_partition, batch_free])
throughout, so no on-device transposes are needed.

ELU is computed as  elu(y)+1 = max(y+1, min(exp(y), 1))  -- one ACT op
(exp with bias -1 reading y+1) plus one fused DVE scalar_tensor_tensor op.
The +1 shifts are absorbed into the next layer's bias via weight row/col
sums, and all biases ride the matmuls as extra contraction rows (hi/lo bf16
split for accuracy) or ACT bias slots.
"""

import numpy as np
import ml_dtypes

import concourse.bass as bass
import concourse.bacc as bacc
import concourse.tile as tile
from concourse import mybir
from concourse.bass_utils import run_bass_kernel_spmd

NCORES = 8
B, XD, PD, D, H, G1 = 512, 32768, 512, 64, 128, 1024
KSH = XD // NCORES   # 4096 contraction slice per core
DL = D // NCORES     # 8 local dims per core
NKDR = KSH // 256    # 16 DoubleRow k-steps for MM1
NG = 4               # MM1 m-groups (each 2 m-tiles of g1) / AllReduce chunks
XSC = 8.0            # fp8 scale on x
WSC = 64.0           # fp8 scale on Wg1
F32 = mybir.dt.float32
BF16 = mybir.dt.bfloat16
F8 = mybir.dt.float8e4
BF = ml_dtypes.bfloat16
F8NP = ml_dtypes.float8_e4m3
AF = mybir.ActivationFunctionType
OP = mybir.AluOpType
DRM = mybir.MatmulPerfMode.DoubleRow

_cache = {}


def _build(sim=False, stage=3):
    nc = bacc.Bacc("TRN2", target_bir_lowering=False, debug=False,
                   num_devices=1 if sim else NCORES)

    # x: chunk q holds DR k-steps 4q..4q+3: [128, kc, i, b] (fp8, scaled x8)
    xt_d = nc.declare_dram_parameter("xt", [4, 128, 4 * 2 * B], F8, isOutput=False)
    # wg1: per (m-pair g, k-octet h): [128, k8, i, m2, 128] (fp8, scaled x64)
    wg1_d = nc.declare_dram_parameter("wg1", [NG, 2, 128, 8 * 2 * 256], F8, isOutput=False)
    f32p_d = nc.declare_dram_parameter("f32p", [128, 12], F32, isOutput=False)
    w2all_d = nc.declare_dram_parameter("w2all", [128, 9216], F8, isOutput=False)

    bfp_d = nc.declare_dram_parameter("bfp", [3, 12288], BF16, isOutput=False)
    w3_d = nc.declare_dram_parameter("w3", [128, DL], BF16, isOutput=False)
    out_d = nc.declare_dram_parameter("out", [384], F32, isOutput=True)

    GROUPS = [[0, 1], [2, 3], [4], [5], [6], [7]]
    aring = [nc.dram_tensor(f"arin{g}", [128, len(grp) * B], F8)
             for g, grp in enumerate(GROUPS)]
    aroutg = [nc.dram_tensor(f"arout{g}", [128, len(grp) * B], F8,
                             addr_space="Shared")
              for g, grp in enumerate(GROUPS)]

    with tile.TileContext(nc) as tc:
        with (
            tc.tile_pool(name="wg1p", bufs=2) as wg1p,
            tc.tile_pool(name="castp", bufs=4) as castp,
            tc.tile_pool(name="consts", bufs=1) as cst,
            tc.tile_pool(name="work", bufs=1) as wk,
            tc.tile_pool(name="ep", bufs=6) as ep,
            tc.tile_pool(name="ap_", bufs=10) as app,
            tc.tile_pool(name="mep", bufs=4) as mep,
        ):
            # ---- startup DMAs in priority order: first MM1 operands,
            # then the rest of x / Wg1, then constants ----
            xtt = [cst.tile([128, 4, 2, B], F8, name=f"xts_{i}") for i in range(4)]
            wg1h = [[wg1p.tile([128, 8, 2, 2, 128], F8,
                               name=f"wg_{g}_{h}") for h in range(2)]
                    for g in range(NG)]
            # interleave so the first matmuls' operands land first
            nc.sync.dma_start(wg1h[0][0][:, 0:2], wg1_d[0, 0][:, 0:1024])
            nc.sync.dma_start(xtt[0][:, 0:2], xt_d[0][:, 0:2048])
            nc.sync.dma_start(wg1h[0][0][:, 2:8], wg1_d[0, 0][:, 1024:4096])
            nc.sync.dma_start(xtt[0][:, 2:4], xt_d[0][:, 2048:4096])
            nc.sync.dma_start(xtt[1][:], xt_d[1])
            nc.sync.dma_start(wg1h[0][1][:], wg1_d[0, 1])
            nc.sync.dma_start(xtt[2][:], xt_d[2])
            nc.sync.dma_start(xtt[3][:], xt_d[3])
            for g in range(1, NG):
                for h in range(2):
                    nc.sync.dma_start(wg1h[g][h][:], wg1_d[g, h])
            w2all = cst.tile([128, 9216], F8)
            nc.sync.dma_start(w2all[:], w2all_d[:])
            wg2sb = w2all[:, 0:4096]
            wht = w2all[:, 4096:8192]
            w2t = w2all[:, 8192:9216]
            # ---- constants: packed into 3 DMAs (f32 pack, bf16 pack, w3)
            # plus the big fp8 stage-2 weights late ----
            f32p = cst.tile([128, 12], F32)
            nc.scalar.dma_start(f32p[:], f32p_d[:])
            bg1c = f32p[:, 0:8]
            cvec4 = f32p[:, 8:12]
            bfp = cst.tile([3, 12288], BF16)
            nc.scalar.dma_start(bfp[:], bfp_d[:])
            b2r = bfp[0:2, 0:512]
            l1b = bfp[:, 512:1536]
            zj = bfp[:, 1536:5632]
            zd = bfp[0:1, 5632:9728]
            l2b = bfp[0:2, 9728:10752]
            ones2 = bfp[0:2, 10752:11264]
            bg1r = bfp[0:1, 11264:12288]
            w3t = cst.tile([128, DL], BF16)

            # warm the exp table set early so the ~2.7us load hides under MM1
            wz0 = cst.tile([128, 1], F32)
            nc.vector.memset(wz0[:], 0.0)
            wz1 = cst.tile([128, 1], F32)
            nc.scalar.activation(wz1[:], wz0[:], AF.Exp)
            neg1 = cst.tile([128, 1], F32)
            nc.vector.memset(neg1[:], -1.0)


            h1g = [wk.tile([128, len(grp) * B], F8, name=f"h1g_{g}")
                   for g, grp in enumerate(GROUPS)]
            e1m = [wk.tile([128, B], BF16, name=f"e1m_{m}") for m in range(8)]
            a1m = [wk.tile([128, B], BF16, name=f"a1m_{m}") for m in range(8)]
            htm = [wk.tile([128, B], BF16, name=f"htm_{mt}") for mt in range(4)]

            with (
                tc.tile_pool(name="ps1", bufs=4, space="PSUM") as ps1,
                tc.tile_pool(name="ps2p", bufs=4, space="PSUM") as ps2p,
            ):
                ps2 = [ps2p.tile([128, B], F32, tag="psmm2", name=f"psmm2_{i}")
                       for i in range(4)]
                # ---- MM1 k-contiguous per m-group + chunked AllReduce.
                # Last pair split into single-m-tile groups so the final
                # (exposed) AllReduce is half the size with half the tail.
                pend = []
                for gi, grp in enumerate(GROUPS):
                    pair = grp[0] // 2
                    gw = len(grp)
                    pg = {m: ps1.tile([128, B], F32, tag="psmm1",
                                      name=f"ps1_{m}") for m in grp}
                    for kt in range(NKDR):
                        wgt = wg1h[pair][kt // 8]
                        ko = kt % 8
                        xsrc = xtt[kt // 4]
                        kc = kt % 4
                        for m in grp:
                            jc = m % 2
                            nc.tensor.matmul(
                                pg[m][:],
                                wgt[:, ko, :, jc],
                                xsrc[:, kc],
                                start=(kt == 0), stop=(kt == NKDR - 1),
                                perf_mode=DRM)
                    for mi, m in enumerate(grp):
                        cp = castp.tile([128, B], F8, tag="cast", name=f"cp_{m}")
                        nc.vector.tensor_scalar(cp[:], pg[m][:],
                                                1.0 / (XSC * WSC),
                                                bg1c[:, m:m + 1],
                                                OP.mult, OP.add)
                        nc.scalar.dma_start(
                            aring[gi][:, mi * B:(mi + 1) * B], cp[:])
                    if stage < 2:
                        continue
                    if sim:
                        nc.gpsimd.dma_start(aroutg[gi][:], aring[gi][:])
                    else:
                        nc.gpsimd.collective_compute(
                            "AllReduce", OP.add,
                            replica_groups=[list(range(NCORES))],
                            ins=[aring[gi][:]], outs=[aroutg[gi][:]],
                        )
                    nc.sync.dma_start(h1g[gi][:], aroutg[gi][:])
                    pend.append((gi, grp))
                    if gi >= 1:
                        pgi, pgrp = pend.pop(0)
                        for mi, m in enumerate(pgrp):
                            h1s = h1g[pgi][:, mi * B:(mi + 1) * B]
                            nc.scalar.activation(e1m[m][:], h1s,
                                                 AF.Exp, bias=neg1[:])
                            nc.vector.scalar_tensor_tensor(a1m[m][:],
                                                           e1m[m][:],
                                                           1.0, h1s,
                                                           OP.min, OP.max)
                    if gi == len(GROUPS) - 1:
                        for pgi, pgrp in pend:
                            for mi, m in enumerate(pgrp):
                                h1s = h1g[pgi][:, mi * B:(mi + 1) * B]
                                nc.scalar.activation(e1m[m][:], h1s,
                                                     AF.Exp, bias=neg1[:])
                                nc.vector.scalar_tensor_tensor(a1m[m][:],
                                                               e1m[m][:],
                                                               1.0, h1s,
                                                               OP.min, OP.max)
                        pend.clear()

                if stage >= 2:
                    nc.sync.dma_start(w3t[:], w3_d[:])
                    # MM2 partials kt 0..6 fill the PE gap while the last AR
                    # chunks land; kt=7 + elu emitted per-mt so htm[0]'s elu
                    # overlaps mt 1-3's matmuls.
                    for kt in range(7):
                        for mt in range(4):
                            nc.tensor.matmul(
                                ps2[mt][:],
                                wg2sb[:, kt * PD + mt * 128:kt * PD + (mt + 1) * 128],
                                a1m[kt][:],
                                start=(kt == 0), stop=False)
                        if kt == 0:
                            for mt in range(4):
                                nc.tensor.matmul(ps2[mt][:],
                                                 b2r[:, mt * 128:(mt + 1) * 128],
                                                 ones2[:], start=False,
                                                 stop=False)
                    for mt in range(4):
                        nc.tensor.matmul(
                            ps2[mt][:],
                            wg2sb[:, 7 * PD + mt * 128:7 * PD + (mt + 1) * 128],
                            a1m[7][:], start=False, stop=True)
                        eh = ep.tile([128, B], BF16, tag="escr", name=f"eh_{mt}")
                        nc.scalar.activation(eh[:], ps2[mt][:], AF.Exp, bias=neg1[:])
                        nc.vector.scalar_tensor_tensor(htm[mt][:], eh[:], 1.0,
                                                       ps2[mt][:], OP.min, OP.max)

            if stage >= 3:
                # ---- stage 2: per-dim nets, joint + marg ----
                rsumJ = wk.tile([128, DL], F32)
                ets4 = wk.tile([128, 4], F32)

                with (
                    tc.tile_pool(name="psL1", bufs=3, space="PSUM") as psL1,
                    tc.tile_pool(name="psL2", bufs=2, space="PSUM") as psL2,
                    tc.tile_pool(name="psm5", bufs=1, space="PSUM") as psm5,
                ):
                    def layer1J(d, nm):
                        """J pre: zj rows first (AR-wait fill), then hWh."""
                        pre = psL1.tile([128, B], F32, tag="psL1", name=f"pL1_{nm}")
                        nc.tensor.matmul(pre[:], l1b[:, d * H:(d + 1) * H],
                                         zj[:, d * B:(d + 1) * B],
                                         start=True, stop=False)
                        for kt in range(4):
                            nc.tensor.matmul(
                                pre[:],
                                wht[:, kt * DL * H + d * H:kt * DL * H + (d + 1) * H],
                                htm[kt][:],
                                start=False, stop=(kt == 3))
                        e = ep.tile([128, B], BF16, tag="escr", name=f"e_{nm}")
                        nc.scalar.activation(e[:], pre[:], AF.Exp, bias=neg1[:])
                        a = app.tile([128, B], BF16, tag="act", name=f"a_{nm}")
                        nc.vector.scalar_tensor_tensor(a[:], e[:], 1.0, pre[:],
                                                       OP.min, OP.max)
                        return pre, a

                    def layer1M(pre, d, nm):
                        """marg pre = joint pre + Wz*(zinv-z), in place."""
                        nc.tensor.matmul(pre[:], l1b[0:1, d * H:(d + 1) * H],
                                         zd[:, d * B:(d + 1) * B],
                                         start=False, stop=True,
                                         skip_group_check=True)
                        e = ep.tile([128, B], BF16, tag="escr", name=f"eM_{nm}")
                        nc.scalar.activation(e[:], pre[:], AF.Exp, bias=neg1[:])
                        a = app.tile([128, B], BF16, tag="act", name=f"aM_{nm}")
                        nc.vector.scalar_tensor_tensor(a[:], e[:], 1.0, pre[:],
                                                       OP.min, OP.max)
                        return a

                    def layer2pair(aJ, aM, d, nm):
                        pre = psL2.tile([128, 2 * B], F32, tag="psL2",
                                        name=f"pL2_{nm}")
                        for half, a in ((0, aJ), (1, aM)):
                            sl = slice(half * B, (half + 1) * B)
                            nc.tensor.matmul(pre[:, sl],
                                             w2t[:, d * H:(d + 1) * H], a[:],
                                             start=True, stop=False)
                            nc.tensor.matmul(pre[:, sl],
                                             l2b[:, d * H:(d + 1) * H],
                                             ones2[:], start=False, stop=True)
                        e = ep.tile([128, 2 * B], BF16, tag="escr2",
                                    name=f"e2_{nm}")
                        nc.scalar.activation(e[:], pre[:], AF.Exp, bias=neg1[:])
                        a2J = app.tile([128, B], BF16, tag="act", name=f"a2J_{nm}")
                        nc.vector.scalar_tensor_tensor(
                            a2J[:], e[:, 0:B], 1.0, pre[:, 0:B], OP.min, OP.max,
                            accum_out=rsumJ[:, d:d + 1])
                        a2M = app.tile([128, B], BF16, tag="act", name=f"a2M_{nm}")
                        nc.vector.scalar_tensor_tensor(
                            a2M[:], e[:, B:], 1.0, pre[:, B:], OP.min, OP.max)
                        return a2M

                    # Software pipeline, all cross-stage deps >= 1 iter old:
                    #   iter: L1J(d) | delta/eM/aM(d-1) | L2J(d-1)+L2M(d-2)
                    #         paired exp | L3(d-3) | mear per 2 dims.
                    preJ, aJs, aMs, aM2s = {}, {}, {}, {}
                    mrow = None
                    for it in range(DL + 3):
                        d = it
                        if d < DL:
                            # L1 joint: zj rows first, then hWh over htm
                            pre = psL1.tile([128, B], F32, tag="psL1",
                                            name=f"pL1_{d}")
                            nc.tensor.matmul(pre[:], l1b[:, d * H:(d + 1) * H],
                                             zj[:, d * B:(d + 1) * B],
                                             start=True, stop=False)
                            for kt in range(4):
                                nc.tensor.matmul(
                                    pre[:],
                                    wht[:, kt * DL * H + d * H:
                                        kt * DL * H + (d + 1) * H],
                                    htm[kt][:], start=False, stop=(kt == 3))
                            preJ[d] = pre
                            eJ = ep.tile([128, B], BF16, tag="escr",
                                         name=f"eJ_{d}")
                            nc.scalar.activation(eJ[:], pre[:], AF.Exp,
                                                 bias=neg1[:])
                            aJ = app.tile([128, B], BF16, tag="act",
                                          name=f"aJ_{d}")
                            nc.vector.scalar_tensor_tensor(aJ[:], eJ[:], 1.0,
                                                           pre[:], OP.min, OP.max)
                            aJs[d] = aJ
                        dm = it - 1
                        if 0 <= dm < DL:
                            # marg delta on J's psum (J reads finished last iter)
                            pre = preJ.pop(dm)
                            nc.tensor.matmul(pre[:], l1b[0:1, dm * H:(dm + 1) * H],
                                             zd[:, dm * B:(dm + 1) * B],
                                             start=False, stop=True,
                                             skip_group_check=True)
                            eM = ep.tile([128, B], BF16, tag="escr",
                                         name=f"eM_{dm}")
                            nc.scalar.activation(eM[:], pre[:], AF.Exp,
                                                 bias=neg1[:])
                            aM = app.tile([128, B], BF16, tag="act",
                                          name=f"aM_{dm}")
                            nc.vector.scalar_tensor_tensor(aM[:], eM[:], 1.0,
                                                           pre[:], OP.min, OP.max)
                            aMs[dm] = aM
                        # L2: J half for dim dm, M half for dim dm-1, one exp
                        dj, dmm = dm, dm - 1
                        havej, havem = 0 <= dj < DL, 0 <= dmm < DL
                        if havej or havem:
                            pre2 = psL2.tile([128, 2 * B], F32, tag="psL2",
                                             name=f"pL2_{it}")
                            if havej:
                                nc.tensor.matmul(pre2[:, 0:B],
                                                 w2t[:, dj * H:(dj + 1) * H],
                                                 aJs.pop(dj)[:],
                                                 start=True, stop=False)
                                nc.tensor.matmul(pre2[:, 0:B],
                                                 l2b[:, dj * H:(dj + 1) * H],
                                                 ones2[:], start=False, stop=True)
                            if havem:
                                nc.tensor.matmul(pre2[:, B:],
                                                 w2t[:, dmm * H:(dmm + 1) * H],
                                                 aMs.pop(dmm)[:],
                                                 start=True, stop=False)
                                nc.tensor.matmul(pre2[:, B:],
                                                 l2b[:, dmm * H:(dmm + 1) * H],
                                                 ones2[:], start=False, stop=True)
                            esl = (slice(0, 2 * B) if (havej and havem)
                                   else slice(0, B) if havej else slice(B, 2 * B))
                            e2 = ep.tile([128, 2 * B], BF16, tag="escr2",
                                         name=f"e2_{it}")
                            nc.scalar.activation(e2[:, esl], pre2[:, esl],
                                                 AF.Exp, bias=neg1[:])
                            if havej:
                                a2J = app.tile([128, B], BF16, tag="act",
                                               name=f"a2J_{dj}")
                                nc.vector.scalar_tensor_tensor(
                                    a2J[:], e2[:, 0:B], 1.0, pre2[:, 0:B],
                                    OP.min, OP.max,
                                    accum_out=rsumJ[:, dj:dj + 1])
                            if havem:
                                a2M = app.tile([128, B], BF16, tag="act",
                                               name=f"a2M_{dmm}")
                                nc.vector.scalar_tensor_tensor(
                                    a2M[:], e2[:, B:], 1.0, pre2[:, B:],
                                    OP.min, OP.max)
                                aM2s[dmm] = a2M
                        de = it - 3
                        if 0 <= de < DL:
                            aM2 = aM2s.pop(de)
                            if de % 2 == 0:
                                mrow = psm5.tile([128, B], F32, tag="psm5",
                                                 name=f"m5_{de}")
                            q = de % 2
                            nc.tensor.matmul(mrow[32 * q:32 * q + 1, :],
                                             w3t[:, de:de + 1],
                                             aM2[:], start=True, stop=True,
                                             skip_group_check=True)
                            if q == 1:
                                j = de // 2
                                mear = mep.tile([128, B], BF16, tag="mescr",
                                                name=f"me_{j}")
                                nc.scalar.activation(
                                    mear[:], mrow[:], AF.Exp,
                                    bias=cvec4[:, j:j + 1],
                                    accum_out=ets4[:, j:j + 1])
                                if j == 1:
                                    nc.sync.dma_start(out_d[0:128],
                                                      ets4[0:64, 0:2])
                                if j == 3:
                                    nc.sync.dma_start(out_d[128:256],
                                                      ets4[0:64, 2:4])

                # joint per-partition dot: jpp[k] = sum_d rsumJ[k,d]*W3[k,d]
                jsc = wk.tile([128, DL], F32)
                jpp = wk.tile([128, 1], F32)
                nc.vector.scalar_tensor_tensor(jsc[:], rsumJ[:], 1.0, w3t[:],
                                               OP.mult, OP.mult, accum_out=jpp[:])
                nc.sync.dma_start(out_d[256:384], jpp[:, 0:1])
    nc.compile()
    return nc


def _hilo(v):
    hi = v.astype(BF)
    lo = (v - hi.astype(np.float32)).astype(BF)
    return hi, lo


def _prep(x, z, perm, Wg1, bg1, Wg2, bg2, Wh, Wz, b1, W2, b2, W3, b3):
    """Build per-core input maps + host-side constants."""
    invperm = np.argsort(perm)
    zinv = z[invperm]                       # [B, D]
    bg2a = bg2 - Wg2.sum(axis=0)            # a1 shift correction
    b1a = b1 - Wh.sum(axis=1)               # [D, H] h~ shift correction
    b2a = b2 - W2.sum(axis=1)               # [D, H] a1 shift correction
    cj = b3 - W3.sum(axis=1)                # [D] a2 shift correction

    b2r_hi, b2r_lo = _hilo(1.0 + bg2a)
    ones2 = np.ones((2, B), BF)

    in_maps = []
    for c in range(NCORES):
        ksl = slice(c * KSH, (c + 1) * KSH)
        dsl = slice(c * DL, (c + 1) * DL)
        # xt[q][p, kc, i, b] = 8*x[b, c*KSH + (4q+kc)*256 + i*128 + p]
        xs = (x[:, ksl] * XSC).astype(F8NP)
        xt = np.ascontiguousarray(
            xs.T.reshape(4, 4, 2, 128, B).transpose(0, 3, 1, 2, 4)
            .reshape(4, 128, 4 * 2 * B))
        # wg1[g, h][p, k8, i, m2, mi] = 64*Wg1_c[(8h+k8)*256 + i*128 + p,
        #                                        (2g+m2)*128 + mi]
        ws = (Wg1[ksl] * WSC).astype(F8NP)
        wq = ws.reshape(NKDR, 2, 128, 8, 128).transpose(3, 0, 1, 2, 4)
        wg1 = np.ascontiguousarray(np.stack([
            np.stack([
                wq[2 * g:2 * g + 2, 8 * h:8 * h + 8]
                .transpose(3, 1, 2, 0, 4).reshape(128, 8 * 2 * 256)
                for h in range(2)])
            for g in range(NG)]))
        bg1c = ((bg1 + 1.0) / 8.0).astype(np.float32).reshape(8, 128).T
        cvec = np.zeros((128, 4), np.float32)
        for j in range(4):
            cvec[0, j] = cj[dsl][2 * j]
            cvec[32, j] = cj[dsl][2 * j + 1]
        f32p = np.concatenate([bg1c, cvec], axis=1).astype(np.float32)
        # wg2[p, kt*PD+m] = Wg2[kt*128+p, m]
        wg2 = np.ascontiguousarray(
            Wg2.reshape(8, 128, PD).transpose(1, 0, 2).reshape(128, 8 * PD)
        ).astype(F8NP)
        bg1row = (64.0 * (bg1 + 1.0)).astype(BF).reshape(1, G1)
        # wh[p, kt*DL*H + d*H+h] = Wh[dg, kt*128+p, h]
        wh = np.ascontiguousarray(
            Wh[dsl].transpose(1, 0, 2).reshape(4, 128, DL * H)
            .transpose(1, 0, 2).reshape(128, 4 * DL * H)).astype(F8NP)
        l1 = np.zeros((3, DL * H), np.float32)
        l1[0] = Wz[dsl].reshape(-1)
        v1hi, v1lo = _hilo((1.0 + b1a[dsl]).reshape(-1))
        l1b = np.stack([l1[0].astype(BF), v1hi, v1lo]).astype(BF)
        zjr = np.zeros((3, DL * B), np.float32)
        zjr[0] = z[:, dsl].T.reshape(-1)
        zjr[1:] = 1.0
        zdr = (zinv[:, dsl] - z[:, dsl]).T.reshape(1, DL * B)
        w2 = np.ascontiguousarray(
            W2[dsl].transpose(1, 0, 2).reshape(H, DL * H)
        ).astype(F8NP)
        v2hi, v2lo = _hilo((1.0 + b2a[dsl]).reshape(-1))
        l2b = np.stack([v2hi, v2lo])
        w3 = np.ascontiguousarray(W3[dsl].T).astype(BF)   # [H, DL]
        bfp = np.zeros((3, 12288), BF)
        bfp[0:2, 0:512] = np.stack([b2r_hi, b2r_lo])
        bfp[:, 512:1536] = l1b
        bfp[:, 1536:5632] = zjr.astype(BF)
        bfp[0:1, 5632:9728] = zdr.astype(BF)
        bfp[0:2, 9728:10752] = l2b
        bfp[0:2, 10752:11264] = ones2
        bfp[0:1, 11264:12288] = bg1row
        w2all = np.concatenate([wg2, wh, w2], axis=1)
        in_maps.append({
            "xt": xt, "wg1": wg1, "f32p": f32p, "w2all": w2all,
            "bfp": bfp, "w3": w3,
        })
    return in_maps, cj


def _combine(results, cj):
    """Host-side final reduction: 64 logs + means."""
    joint_sum = 0.0
    log_sum = 0.0
    for c in range(NCORES):
        o = results[c]["out"].astype(np.float64)
        joint_sum += o[256:384].sum() / B
        em = np.concatenate([o[0:128].reshape(64, 2),
                             o[128:256].reshape(64, 2)], axis=1)
        ets = em[0:64:32, :].T.reshape(-1)    # dims 0..7 local order
        log_sum += np.log(ets / B).sum()
    joint_sum += cj.astype(np.float64).sum()
    mi_sum = joint_sum - log_sum
    return np.float32(-mi_sum / D)


def kernel(x, z, perm, Wg1, bg1, Wg2, bg2, Wh, Wz, b1, W2, b2, W3, b3):
    args = (x, z, perm, Wg1, bg1, Wg2, bg2, Wh, Wz, b1, W2, b2, W3, b3)
    args = tuple(np.asarray(a) for a in args)
    in_maps, cj = _prep(*args)
    if "nc" not in _cache:
        _cache["nc"] = _build()
    r = run_bass_kernel_spmd(_cache["nc"], in_maps, list(range(NCORES)))
    return _combine(r.results, cj)

